# revision 8
# baseline (speedup 1.0000x reference)
"""Distributed Trainium2 kernel for nn_CompareLoss (8 NeuronCores), v2.

Math (validated against the reference):
  z = [strong; weak]  (2B x D), s = z / ||z||  (row-normalized)
  logits(i,j) = (s_i . s_j) / tau,  pos_i = logits(i, B+i) = logits(B+i, i)
  Every row r of the similarity matrix contributes  ln(S_r) - pos_r  where
    S_r = exp(pos_r) + sum_{j in C(r)} exp(logits(r, j))
  with column set C(r):
    - "positive" rows (strong_i / weak_{B+i}, i < P): C = all 2N negative rows
    - "negative" rows (i >= P):                       C = the P strong-positive rows
  loss = (sum over all 2B rows) / (2B).

Sharding: data-parallel over the pair index i (core c owns 256 positives and
256 negatives -> 1024 rows). Full column set shipped per core, feature-major,
with the core's own row blocks rotated to the front of each region so one
SPMD program serves all 8 cores. No collectives (8-rank collective floor
~7-20us > host-summing 8 scalars).

v2 changes vs the 80.7us baseline:
  * All similarity / sum-sq matmuls run fp8(e4m3) in DoubleRow perf mode:
    one matmul does the whole K=256 contraction at 2 rows/cycle (PE busy
    halves). Offline fp8 pipeline check vs float64 ref: rel err 7.7e-6.
  * Contiguous column layout [sp | wp_c | sn_c | wn_c | sn_rest | wn_rest]:
    G1 (everything the M2 jobs + pos + all lhsT tiles need) = cols 0:2816,
    M1 rhs = cols 2304:6400, M2 rhs = cols 0:2048 - all contiguous slices,
    so sum-sq batches and rn writes are single wide ops.
  * Column norms land REPLICATED across partitions: the sum-sq ones-matmul
    uses an all-ones [128,2,128] fp8 lhsT, so psum[p, c] = ssq(col c) for
    every p. rn = exp(-0.5*ln(ssq) + 0.5*ln(1/tau)) then runs as [128, w]
    ACT ops (Ln+Exp share one table set with the main Exps -> no table
    churn) writing rnb directly - no DRAM round trip, no broadcast DMAs.
  * Input DMA is chunked+chained in consumption order so squares start ~2us
    after launch instead of 12us.
  Mains: 12 jobs of [128,2048]: 4 DoubleRow matmuls + one Exp with fused
  row-sum (activation accum_out). ln(S)-pos is reduced on-chip to a single
  f32 partial per core; the host adds 8 partials and divides by 2B.
"""

import numpy as np

B = 4096
D = 256
P = 2048
NCORES = 8
IC = P // NCORES          # 256 pair-indices per core (per pos/neg half)
NCOL = 3 * P + IC         # 6400 columns

# column layout: [sp(2048,rot) | wp_c(256) | sn_c(256) | wn_c(256)
#                 | sn_rest(1792,rot) | wn_rest(1792,rot)]
OFF_SP = 0
OFF_WP = 2048
OFF_SNC = 2304
OFF_WNC = 2560
OFF_SNR = 2816
OFF_WNR = 4608
G1W = 2816                # cols 0:2816 unblock pos + lhsT + M2 rhs
M1_RHS = 2304             # all-negative columns 2304:6400 (4096 wide)

# lhsT column offsets for the 8 m-tiles (128 rows each)
LHS_OFF = [OFF_SP, OFF_SP + 128, OFF_WP, OFF_WP + 128,
           OFF_SNC, OFF_SNC + 128, OFF_WNC, OFF_WNC + 128]
POS_COL = [0, 1, 0, 1, 2, 3, 2, 3]   # pos i-tile used by each m-tile

# sum-sq batches: (start, width, list of (chunk_start, chunk_w) matmul slices)
SS_BATCHES = [
    (0, 2048),        # ssA: sp
    (2048, 768),      # ssB: wp_c + sn_c + wn_c
    (2816, 2048),     # ssC: sn_rest + wn_rest[:256]
    (4864, 1536),     # ssD: rest
]

_CACHE: dict = {}


def _build_nc():
    import concourse.bacc as bacc
    import concourse.tile as tile
    from concourse import mybir

    f32 = mybir.dt.float32
    f16 = mybir.dt.float16
    f8 = mybir.dt.float8e4
    EXP = mybir.ActivationFunctionType.Exp
    LN = mybir.ActivationFunctionType.Ln
    AX = mybir.AxisListType.X
    ADD = mybir.AluOpType.add
    DR = mybir.MatmulPerfMode.DoubleRow

    nc = bacc.Bacc("TRN2", target_bir_lowering=False, debug=False,
                   num_devices=NCORES)
    zt_d = nc.dram_tensor("zt", [D, NCOL], f16, kind="ExternalInput")
    tp_d = nc.dram_tensor("temp", [1, 1], f32, kind="ExternalInput")
    out_d = nc.dram_tensor("out", [1, 1], f32, kind="ExternalOutput")

    with tile.TileContext(nc) as tc:
        with (
            tc.tile_pool(name="const", bufs=1) as constp,
            tc.tile_pool(name="big", bufs=1) as bigp,
            tc.tile_pool(name="work", bufs=1) as workp,
            tc.tile_pool(name="lnb", bufs=2) as lnp,
            tc.tile_pool(name="esc", bufs=2) as escp,
            tc.tile_pool(name="ps", bufs=2, space="PSUM") as psp,
        ):
            # ---------------- load z^T chunked in consumption order --------
            from concourse.tile_rust import add_dep_helper
            zt0 = bigp.tile([128, NCOL], f16)        # features 0:128
            zt1 = bigp.tile([128, NCOL], f16)        # features 128:256
            dma_groups = [
                [(0, 1024)], [(1024, 1024)], [(2048, 768)],
                [(2816, 1024)], [(3840, 1024)], [(4864, 1536)],
            ]
            prev = []
            for grp in dma_groups:
                cur = []
                for s, w in grp:
                    cur.append(nc.sync.dma_start(zt0[:, s:s + w],
                                                 zt_d[0:128, s:s + w]))
                    cur.append(nc.sync.dma_start(zt1[:, s:s + w],
                                                 zt_d[128:D, s:s + w]))
                for a in cur:
                    for b in prev:
                        add_dep_helper(a.ins, b.ins, sync=True,
                                       reason="dma group ordering")
                prev = cur

            # ---------------- constants ----------------
            ones8_bc = constp.tile([128, 2, 128], f8)   # replicating col-sum
            nc.gpsimd.memset(ones8_bc[:], 1.0)
            # flat col-sum lhsT for pos: [128,2,1] slice of a [128,2,16]
            # tile so the k-pair dim keeps a 16-element step (dual-fp8
            # LDWEIGHTS requires outer free steps to be 16-aligned)
            ones8_k16 = constp.tile([128, 2, 16], f8)
            nc.gpsimd.memset(ones8_k16[:], 1.0)
            ones8_k = ones8_k16[:, :, 0:1]
            ones16_1 = constp.tile([1, 128], f16)
            nc.gpsimd.memset(ones16_1[:], 1.0)
            ident16 = constp.tile([1, 1], f16)
            nc.gpsimd.memset(ident16[:], 1.0)
            ones_k = constp.tile([128, 1], f32)         # final f32 total-sum
            nc.gpsimd.memset(ones_k[:], 1.0)

            tsb = constp.tile([1, 1], f32)
            nc.sync.dma_start(tsb[:], tp_d[:])
            invt = constp.tile([1, 1], f32)
            nc.vector.reciprocal(invt[:], tsb[:])
            ln_invt = constp.tile([1, 1], f32)
            nc.scalar.activation(ln_invt[:], invt[:], LN)
            half_ln_invt = constp.tile([1, 1], f32)
            nc.scalar.mul(half_ln_invt[:], ln_invt[:], 0.5)
            # broadcast 0.5*ln(1/tau) to [128,1] via a K=1 matmul
            hli16 = constp.tile([1, 1], f16)
            nc.vector.tensor_copy(hli16[:], half_ln_invt[:])
            bias_ps = psp.tile([128, 1], f32, tag="ps")
            nc.tensor.matmul(bias_ps[:], ones16_1[:], hli16[0:1, 0:1],
                             start=True, stop=True)
            bias_bc = constp.tile([128, 1], f32)     # 0.5*ln(1/tau) everywhere
            nc.vector.tensor_copy(bias_bc[:], bias_ps[:])

            # ---------------- normalization machinery ----------------
            ztn = bigp.tile([128, 2, NCOL], f8)      # normalized, DR-paired
            rnb = bigp.tile([128, NCOL], f16)        # 1/(sqrt(tau)*||z_j||)

            def squares(eng, s, w, sq):
                """sq[128,2,w] fp8 <- elementwise squares of zt cols s:s+w."""
                eng.tensor_mul(sq[:, 0, :w], zt0[:, s:s + w], zt0[:, s:s + w])
                eng.tensor_mul(sq[:, 1, :w], zt1[:, s:s + w], zt1[:, s:s + w])

            def ss_batch(s, w, sqs):
                """replicated column sum-sq for cols s:s+w -> rn into rnb.
                sqs: list of (sq_tile, off, cw) covering the batch."""
                ps = psp.tile([128, 2048], f32, tag="ps", name=f"ss{s}")
                for sq, off, cw in sqs:
                    for c0 in range(0, cw, 512):
                        cs = min(512, cw - c0)
                        nc.tensor.matmul(ps[:, off + c0:off + c0 + cs],
                                         ones8_bc[:],
                                         sq[:, :, c0:c0 + cs],
                                         start=True, stop=True, perf_mode=DR)
                lnb = lnp.tile([128, 2048], f32, tag="ln", name=f"ln{s}")
                nc.scalar.activation(lnb[:, :w], ps[:, :w], LN)
                nc.scalar.activation(rnb[:, s:s + w], lnb[:, :w], EXP,
                                     scale=-0.5, bias=bias_bc[:, 0:1])

            def norm(eng, s, w):
                eng.tensor_mul(ztn[:, 0, s:s + w], zt0[:, s:s + w],
                               rnb[:, s:s + w])
                eng.tensor_mul(ztn[:, 1, s:s + w], zt1[:, s:s + w],
                               rnb[:, s:s + w])

            # --- G1: squares -> ssA/ssB -> rn -> normalize ---
            sqA = workp.tile([128, 2, 2048], f8, tag="sqA")
            for c0 in range(0, 2048, 512):
                squares(nc.vector, c0, 512, sqA[:, :, c0:c0 + 512])
            ss_batch(0, 2048, [(sqA, 0, 2048)])
            sqB = workp.tile([128, 2, 768], f8, tag="sqB")
            squares(nc.gpsimd, 2048, 768, sqB)
            ss_batch(2048, 768, [(sqB, 0, 768)])
            # G1 normalize: sp on DVE, tail on gpsimd
            norm(nc.vector, 0, 1024)
            norm(nc.vector, 1024, 1024)
            norm(nc.gpsimd, 2048, 768)

            # ---------------- pos logits (all columns in G1) ---------------
            # pos pairs: (sp_c cols 0:256) x (wp cols 2048:2304) and
            #            (sn_c 2304:2560) x (wn_c 2560:2816)
            pos_ps = psp.tile([1, 512], f32, tag="ps")
            for half, (ca, cb) in enumerate([(OFF_SP, OFF_WP),
                                             (OFF_SNC, OFF_WNC)]):
                pr = workp.tile([128, 2, IC], f8, tag=f"pr{half}")
                nc.vector.tensor_mul(pr[:], ztn[:, :, ca:ca + IC],
                                     ztn[:, :, cb:cb + IC])
                o = half * 256
                nc.tensor.matmul(pos_ps[0:1, o:o + IC], ones8_k, pr[:],
                                 start=True, stop=True, perf_mode=DR)
            pos_sb = constp.tile([1, 512], f32)
            nc.vector.tensor_copy(pos_sb[:], pos_ps[:])
            pos16 = constp.tile([1, 512], f16)
            nc.vector.tensor_copy(pos16[:], pos_sb[:])

            # ---------------- main similarity jobs ----------------
            ACC = constp.tile([128, 12], f32)

            def main_job(mt, j, acccol):
                off = LHS_OFF[mt]
                js = (M1_RHS if mt < 4 else 0) + j * 2048
                ps = psp.tile([128, 2048], f32, tag="ps", name=f"mm{acccol}")
                for h in range(4):
                    c0 = js + h * 512
                    nc.tensor.matmul(ps[:, h * 512:(h + 1) * 512],
                                     ztn[:, :, off:off + 128],
                                     ztn[:, :, c0:c0 + 512],
                                     start=True, stop=True, perf_mode=DR)
                esc = escp.tile([128, 2048], f16, tag="esc",
                                name=f"esc{acccol}")
                nc.scalar.activation(esc[:], ps[:], EXP,
                                     accum_out=ACC[:, acccol:acccol + 1])

            # M2 jobs (need only G1 columns); G2 squares/sum-sq interleave
            sqC = workp.tile([128, 2, 2048], f8, tag="sqC")
            for c0 in range(0, 2048, 1024):     # G2a squares split engines
                squares(nc.vector, 2816 + c0, 1024, sqC[:, :, c0:c0 + 1024])
            sqD = workp.tile([128, 2, 1536], f8, tag="sqD")
            squares(nc.gpsimd, 4864, 768, sqD[:, :, 0:768])
            squares(nc.gpsimd, 5632, 768, sqD[:, :, 768:1536])

            main_job(4, 0, 8)
            main_job(5, 0, 9)
            ss_batch(2816, 2048, [(sqC, 0, 2048)])
            main_job(6, 0, 10)
            ss_batch(4864, 1536, [(sqD, 0, 1536)])
            main_job(7, 0, 11)
            # G2 normalize split across engines
            norm(nc.vector, 2816, 1024)
            norm(nc.gpsimd, 3840, 1024)
            norm(nc.vector, 4864, 768)
            norm(nc.gpsimd, 5632, 768)

            # transpose pos to per-partition layout via [1,128]x[1,1] matmuls
            P_mat = constp.tile([128, 8], f32)
            for t in range(4):
                pos_t = psp.tile([128, 1], f32, tag="ps", name=f"pt{t}")
                nc.tensor.matmul(pos_t[:], pos16[0:1, t * 128:(t + 1) * 128],
                                 ident16[0:1, 0:1], start=True, stop=True)
                for col in range(8):
                    if POS_COL[col] == t:
                        nc.vector.tensor_copy(P_mat[:, col:col + 1], pos_t[:])
            E_mat = constp.tile([128, 8], f32)
            nc.scalar.activation(E_mat[:], P_mat[:], EXP)

            # M1 jobs
            for j in range(2):
                for mt in range(4):
                    main_job(mt, j, mt * 2 + j)

            # ---------------- reduce & finish ----------------
            RS = constp.tile([128, 8], f32)
            nc.vector.tensor_reduce(
                RS[:, 0:4], ACC[:, 0:8].rearrange("p (m j) -> p m j", j=2),
                axis=AX, op=ADD)
            nc.vector.tensor_copy(RS[:, 4:8], ACC[:, 8:12])
            S_mat = constp.tile([128, 8], f32)
            nc.vector.tensor_add(S_mat[:], RS[:], E_mat[:])
            LnS = constp.tile([128, 8], f32)
            nc.scalar.activation(LnS[:], S_mat[:], LN)
            Dif = constp.tile([128, 8], f32)
            nc.vector.tensor_sub(Dif[:], LnS[:], P_mat[:])
            part = constp.tile([128, 1], f32)
            nc.vector.tensor_reduce(part[:], Dif[:], axis=AX, op=ADD)
            tot_ps = psp.tile([1, 1], f32, tag="ps")
            nc.tensor.matmul(tot_ps[0:1, 0:1], ones_k[:], part[:],
                             start=True, stop=True)
            out_sb = constp.tile([1, 1], f32)
            nc.vector.tensor_copy(out_sb[:], tot_ps[:])
            nc.sync.dma_start(out_d[:], out_sb[:])

    nc.compile()
    return nc


def get_nc():
    if "nc" not in _CACHE:
        _CACHE["nc"] = _build_nc()
    return _CACHE["nc"]


def make_in_maps(strong: np.ndarray, weak: np.ndarray, temp: np.ndarray):
    """Host-side sharding: slice + rotate + transpose (pure data movement)."""
    sp_all = strong[0:P]
    sn_all = strong[P:B]
    wn_all = weak[P:B]
    in_maps = []
    for c in range(NCORES):
        r = c * IC
        sp = np.roll(sp_all, -r, axis=0)          # own sp_c first
        snr = np.roll(sn_all, -r, axis=0)         # own sn_c first
        wnr = np.roll(wn_all, -r, axis=0)         # own wn_c first
        wp = weak[r:r + IC]
        zt = np.ascontiguousarray(
            np.concatenate([sp, wp, snr[0:IC], wnr[0:IC],
                            snr[IC:], wnr[IC:]], axis=0).T.astype(np.float16))
        in_maps.append({"zt": zt, "temp": temp})
    return in_maps


def kernel(inputs, strong_inputs, targets, num_pos, temperature):
    assert int(num_pos) == P
    strong = np.ascontiguousarray(np.asarray(strong_inputs, dtype=np.float32))
    weak = np.ascontiguousarray(np.asarray(inputs, dtype=np.float32))
    temp = np.asarray(temperature, dtype=np.float32).reshape(1, 1)

    from concourse.bass_utils import run_bass_kernel_spmd

    nc = get_nc()
    in_maps = make_in_maps(strong, weak, temp)
    res = run_bass_kernel_spmd(nc, in_maps, core_ids=list(range(NCORES)))
    total = sum(float(np.asarray(r["out"]).reshape(-1)[0])
                for r in res.results)
    return np.float32(total / (2 * B))
